# revision 4
# baseline (speedup 1.0000x reference)
"""GRU-over-neighbors GNN message passing on 8 Trainium2 NeuronCores.

Sharding (per spec hint): data-parallel over nodes — neigh_idx/output rows
split across the 8 cores (6256 rows each, padded 50000->50048); feat and the
small GRU/linear/PReLU params are replicated on every core so the neighbor
gather is core-local (no collectives). Executed as one SPMD program via
jax.pmap; results are concatenated and trimmed on host.
"""

import numpy as np

N, K, D, OUT = 50000, 16, 128, 128
NC = 8
PC = 6256           # rows per core; 8 * 6256 = 50048
NPAD = NC * PC

_cache = {}


def _get_fn():
    if "fn" in _cache:
        return _cache["fn"]
    import jax
    import jax.numpy as jnp

    def fwd(feat_full, self_rows, ni_rows, W_ih, W_hh, b_ih, b_hh,
            W_self, W_neigh, alpha):
        # neighbor mailbox gather, core-local: [PC, K, D]
        m = jnp.take(feat_full, ni_rows, axis=0)

        def step(h, x):
            gi = x @ W_ih.T + b_ih
            gh = h @ W_hh.T + b_hh
            gi_r, gi_z, gi_n = jnp.split(gi, 3, axis=-1)
            gh_r, gh_z, gh_n = jnp.split(gh, 3, axis=-1)
            r = jax.nn.sigmoid(gi_r + gh_r)
            z = jax.nn.sigmoid(gi_z + gh_z)
            n = jnp.tanh(gi_n + r * gh_n)
            return (1.0 - z) * n + z * h, None

        h0 = jnp.zeros((m.shape[0], D), dtype=m.dtype)
        hn, _ = jax.lax.scan(step, h0, jnp.swapaxes(m, 0, 1))
        rst = self_rows @ W_self.T + hn @ W_neigh.T
        return jnp.where(rst >= 0, rst, alpha * rst)

    devs = jax.devices()[:NC]
    fn = jax.pmap(
        fwd,
        in_axes=(None, 0, 0, None, None, None, None, None, None, None),
        devices=devs,
    )
    _cache["fn"] = fn
    return fn


def kernel(**inputs) -> np.ndarray:
    feat = np.asarray(inputs["feat"], np.float32)
    ni = np.asarray(inputs["neigh_idx"], np.int32)

    pad = NPAD - N
    ni_p = np.concatenate([ni, np.zeros((pad, K), ni.dtype)], axis=0)
    self_p = np.concatenate([feat, np.zeros((pad, D), feat.dtype)], axis=0)

    fn = _get_fn()
    out = fn(
        feat,
        self_p.reshape(NC, PC, D),
        ni_p.reshape(NC, PC, K),
        np.asarray(inputs["W_ih"], np.float32),
        np.asarray(inputs["W_hh"], np.float32),
        np.asarray(inputs["b_ih"], np.float32),
        np.asarray(inputs["b_hh"], np.float32),
        np.asarray(inputs["W_self"], np.float32),
        np.asarray(inputs["W_neigh"], np.float32),
        np.asarray(inputs["alpha"], np.float32),
    )
    return np.asarray(out).reshape(NPAD, OUT)[:N].astype(np.float32)
